# revision 20
# baseline (speedup 1.0000x reference)
"""Trainium2 Bass kernel for nn_MemoryWriter (scatter_memory).

Math (see reference):
    w        = where(gate > 0.01, gate * 0.1, 0)            [B]
    contrib  (q_a, v_a, w_a) scattered to slots top_indices[a, :]
    upd_k[s] = sum_j w_j q_j / (counts>0 ? counts : 1), counts = sum_j w_j
    out_k    = mem_k + 0.9 * mom_k + (1 - 0.9) * upd_k      (mom is zeros)

Because upd is a ratio, the 0.1 UPDATE_RATE cancels between numerator and
denominator; we use raw gated gate values g = gate * (gate > 0.01) as weights
and apply the single (1 - momentum) factor at the end.  counts are either 0
or >= 0.01, and a zero count implies an exactly-zero numerator, so the
denominator select becomes rec01 = 1 / (max(counts, tiny) / (1-momentum)).

Sharding: slot dimension across 8 cores (8192 slots each).  The host performs
the contribution routing that the all-to-all performs in a real distributed
setting (the sharding hint: "route each (query, slot_idx) contribution to the
owning device (all-to-all on flattened top_indices)"): each core receives a
dense buffer of its routed contribution rows, packed [q | v | 1 | 1], grouped
by 128-slot tile.  Tiles are padded to a 32-row granularity and grouped into
capacity classes so the padding stays small.  The device then, per slot tile:
  - builds a weighted one-hot lhsT on the fly: (iota == s) * w, with s = -1
    sentinel on padding rows,
  - one PE float32r matmul per (tile, fragment) incidence accumulates
    [K-upd | V-upd | counts | counts] into a per-tile PSUM slice,
  - the ACT engine scales by (1-momentum)/counts, and DVE/GpSimd add the
    memory-table tile.
"""

import numpy as np

# ---- problem constants (hardcoded per contest contract) --------------------
N_SLOTS = 65536
DIM = 128
B = 4096
K = 8
NCORES = 8
SPC = N_SLOTS // NCORES      # slots per core = 8192
NT = SPC // 128              # slot tiles per core = 64
P = 128
EL = 258                     # packed row: [q(128) | v(128) | 1 | 1] f32
GATE_THRESH = 0.01
MOMENTUM = 0.9
UPD = float(np.float32(1.0) - np.float32(MOMENTUM))  # exactly as fp32 computes it
INV_UPD = float(np.float32(1.0) / np.float32(UPD))
USE_BF16 = True              # bf16 contribution path (1 cyc/row matmul)
USE_F32R = not USE_BF16      # float32r matmul (1 cyc/row at even N>=256)

_BUILD_CACHE = {}


def build_nc(struct):
    """Build the per-core Bass program.

    struct: (classes, incid) where classes is a tuple of
    (cap, ntiles, row_offset) DMA groups of the routed buffer and incid is a
    per slot-tile tuple of (col, class_id, pos, cap, start, stop) incidences.
    """
    import concourse.bacc as bacc
    import concourse.tile as tile
    from concourse import mybir
    from contextlib import ExitStack

    classes, incid = struct
    f32 = mybir.dt.float32
    f32r = mybir.dt.float32r
    Alu = mybir.AluOpType
    Act = mybir.ActivationFunctionType

    NCOL = sum(len(v) for v in incid)
    TOTROWS = sum(cap * nt for cap, nt, _ in classes)
    mmdt = mybir.dt.float16 if USE_BF16 else (f32r if USE_F32R else f32)

    nc = bacc.Bacc("TRN2", target_bir_lowering=False, debug=False)

    mem_kv = nc.dram_tensor("mem_kv", [SPC, 2 * DIM], f32, kind="ExternalInput")
    routed = nc.dram_tensor("routed", [TOTROWS, EL], mmdt, kind="ExternalInput")
    sv = nc.dram_tensor("sv", [P, NCOL], f32, kind="ExternalInput")
    wb = nc.dram_tensor("wb", [P, NCOL], f32, kind="ExternalInput")
    out_kv = nc.dram_tensor("out_kv", [SPC, 2 * DIM], f32, kind="ExternalOutput")

    G = 8                    # slot tiles per DMA group (512KB per table)
    PG = 4                   # slot tiles per PSUM group (4 banks)

    with tile.TileContext(nc) as tc, ExitStack() as ctx:
        const = ctx.enter_context(tc.tile_pool(name="const", bufs=1))
        gpool = ctx.enter_context(tc.tile_pool(name="gath", bufs=1))
        wpool = ctx.enter_context(tc.tile_pool(name="work", bufs=8))
        spool = ctx.enter_context(tc.tile_pool(name="small", bufs=8))
        upool = ctx.enter_context(tc.tile_pool(name="upd", bufs=4))
        pspool = ctx.enter_context(tc.tile_pool(name="ps", bufs=2, space="PSUM"))

        # constants / routing metadata
        iota_t = const.tile([P, 128], f32)
        nc.gpsimd.iota(
            iota_t[:], pattern=[[1, 128]], channel_multiplier=0,
            allow_small_or_imprecise_dtypes=True,
        )
        sv_t = const.tile([P, NCOL], f32)
        nc.sync.dma_start(sv_t[:], sv[:, :])
        wb_t = const.tile([P, NCOL], f32)
        nc.sync.dma_start(wb_t[:], wb[:, :])

        # w = gate * (gate > 0.01), per fragment column
        msk_t = const.tile([P, NCOL], f32)
        nc.vector.tensor_scalar(msk_t[:], wb_t[:], GATE_THRESH, None, op0=Alu.is_gt)
        w_t = const.tile([P, NCOL], f32)
        nc.vector.tensor_tensor(w_t[:], wb_t[:], msk_t[:], op=Alu.mult)

        # routed contribution rows, by capacity class.  Chunked loads so
        # compute can start before the whole buffer lands.
        clsbuf = []
        for ci, (cap, ntl, roff) in enumerate(classes):
            buf = gpool.tile([P, ntl * EL], mmdt, tag=f"cls{ci}")
            b3 = buf[:].rearrange("p (t e) -> p t e", e=EL)
            CH = max(1, (8 * 128) // cap)       # ~1K rows per chunk
            pos = 0
            while pos < ntl:
                bs = min(CH, ntl - pos)
                src = routed[roff + pos * cap: roff + (pos + bs) * cap, :]
                nc.sync.dma_start(
                    b3[0:cap, pos:pos + bs, :],
                    src.rearrange("(t p) e -> p t e", p=cap),
                )
                pos += bs
            clsbuf.append(b3)

        for g in range(NT // G):
            r0 = g * G * 128
            upd = upool.tile([P, G * 256], f32, tag="upd")
            upd3 = upd[:].rearrange("p (i c) -> p i c", c=256)

            for pg in range(G // PG):
                # PSUM: 4 tiles x one [512] bank each; [q|v|c|c] at i*512+0..258
                ps = pspool.tile([P, PG * 512], f32, tag="ps")
                ps3 = ps[:].rearrange("p (i c) -> p i c", c=512)
                for i in range(PG):
                    t = g * G + pg * PG + i
                    for col, ci, tpos, cap, st, sp in incid[t]:
                        oh = wpool.tile([P, 128], mmdt, tag="oh")
                        nc.vector.tensor_scalar(
                            oh[0:cap, :], iota_t[0:cap, :],
                            sv_t[0:cap, col:col + 1], w_t[0:cap, col:col + 1],
                            op0=Alu.is_equal, op1=Alu.mult,
                        )
                        nc.tensor.matmul(
                            ps[:, i * 512:i * 512 + EL],
                            lhsT=oh[0:cap, :],
                            rhs=clsbuf[ci][0:cap, tpos, :],
                            start=st, stop=sp,
                        )
                # epilogue for the 4-tile group
                cnt = ps3[:, :, 256:257]                      # [P, 4, 1]
                den = spool.tile([P, PG], f32, tag="den")
                nc.vector.tensor_scalar(den[:], cnt, 1e-30, INV_UPD,
                                        op0=Alu.max, op1=Alu.mult)
                rec01 = spool.tile([P, PG], f32, tag="rec01")
                nc.vector.reciprocal(rec01[:], den[:])

                # upd = psum * rec01 (per-partition scale) on the ACT engine
                for i in range(PG):
                    nc.scalar.activation(
                        upd3[:, pg * PG + i, :], ps3[:, i, 0:256], Act.Copy,
                        scale=rec01[:, i:i + 1],
                    )

            # accumulate the memory tables into upd via DMA compute, then
            # store the finished tiles
            mkv = mem_kv[r0:r0 + G * 128, :].rearrange("(a p) d -> p a d", p=P)
            okv = out_kv[r0:r0 + G * 128, :].rearrange("(a p) d -> p a d", p=P)
            nc.gpsimd.dma_start(upd3[:, :, :], mkv, accum_op=Alu.add)
            nc.sync.dma_start(okv, upd3[:, :, :])

    nc.compile()
    return nc


def prepare_inputs(inputs):
    """Host-side routing (the all-to-all stand-in): bucket contributions by
    (core, slot-tile) and materialize each core's routed row buffer."""
    mkv = np.concatenate([
        np.asarray(inputs["memory_keys"], dtype=np.float32),
        np.asarray(inputs["memory_values"], dtype=np.float32),
    ], axis=1)
    q = np.asarray(inputs["write_query"], dtype=np.float32)
    v = np.asarray(inputs["write_value"], dtype=np.float32)
    gate = np.asarray(inputs["gate_weights"], dtype=np.float32)
    ti = np.asarray(inputs["top_indices"]).astype(np.int64).reshape(-1)

    qv = np.zeros((B, EL), dtype=np.float32)
    qv[:, 0:DIM] = q
    qv[:, DIM:2 * DIM] = v
    qv[:, 2 * DIM] = 1.0
    qv[:, 2 * DIM + 1] = 1.0   # second ones column: fp32r needs even width

    a = np.arange(B * K, dtype=np.int64) // K
    key = ti >> 7                       # global 128-slot tile id [0, 512)
    order = np.argsort(key, kind="stable")
    ks = key[order]
    a_s = a[order]
    s_s = (ti & 127)[order].astype(np.float32)
    cnt = np.bincount(key, minlength=NCORES * NT)
    starts = np.zeros(NCORES * NT + 1, dtype=np.int64)
    starts[1:] = np.cumsum(cnt)

    # Shared structure: per tile, fragments of <=128 rows sized by the max
    # count across cores, rounded up to 32-row granularity and grouped into
    # capacity classes.
    cnt2 = cnt.reshape(NCORES, NT)
    cnt_max = cnt2.max(axis=0)
    frags = []                          # (tile, frag_idx, cap)
    for t in range(NT):
        n = int(cnt_max[t])
        fi = 0
        while n > 128:
            frags.append((t, fi, 128))
            n -= 128
            fi += 1
        frags.append((t, fi, max(32, -(-n // 32) * 32)))

    caps = sorted({cap for _, _, cap in frags})
    classes = []
    frag_place = {}                     # (tile, fi) -> (col, ci, pos, cap)
    col = 0
    roff = 0
    for ci, cap in enumerate(caps):
        members = [f for f in frags if f[2] == cap]
        for pos, (t, fi, _) in enumerate(members):
            frag_place[(t, fi)] = (col, ci, pos, cap)
            col += 1
        classes.append((cap, len(members), roff))
        roff += cap * len(members)
    ncol = col
    totrows = roff

    incid = []
    for t in range(NT):
        lst = sorted(
            [v2 for (tt, fi), v2 in frag_place.items() if tt == t],
            key=lambda x: x[0],
        )
        n = len(lst)
        incid.append(tuple(
            (c, ci, pos, cap, i == 0, i == n - 1)
            for i, (c, ci, pos, cap) in enumerate(lst)
        ))
    incid = tuple(incid)
    struct = (tuple(classes), incid)

    if USE_BF16:
        qv = qv.astype(np.float16)
    in_maps = []
    for c in range(NCORES):
        routed = np.zeros((totrows, EL), dtype=qv.dtype)
        sv_core = np.full((P, ncol), -1.0, dtype=np.float32)
        wb_core = np.zeros((P, ncol), dtype=np.float32)
        for t in range(NT):
            n_c = int(cnt2[c, t])
            src0 = int(starts[c * NT + t])
            done = 0
            for (cc, ci, pos, cap, st, sp) in incid[t]:
                take = min(cap, n_c - done)
                if take <= 0:
                    break
                rows = slice(src0 + done, src0 + done + take)
                cap_, ntl_, roff_ = classes[ci]
                base = roff_ + pos * cap
                routed[base:base + take] = qv[a_s[rows]]
                prt = np.arange(0, take)
                sv_core[prt, cc] = s_s[rows]
                wb_core[prt, cc] = gate[a_s[rows]]
                done += take
        in_maps.append({
            "mem_kv": mkv[c * SPC:(c + 1) * SPC],
            "routed": routed,
            "sv": np.ascontiguousarray(sv_core),
            "wb": np.ascontiguousarray(wb_core),
        })
    return in_maps, struct


def kernel(**inputs):
    from concourse.bass_utils import run_bass_kernel_spmd

    in_maps, struct = prepare_inputs(inputs)
    if struct not in _BUILD_CACHE:
        _BUILD_CACHE[struct] = build_nc(struct)
    nc = _BUILD_CACHE[struct]

    res = run_bass_kernel_spmd(nc, in_maps, core_ids=list(range(NCORES)))
    out_kv = np.concatenate([res.results[c]["out_kv"] for c in range(NCORES)], axis=0)
    out_k = np.ascontiguousarray(out_kv[:, 0:DIM])
    out_v = np.ascontiguousarray(out_kv[:, DIM:2 * DIM])

    km = np.asarray(inputs["key_momentum"], dtype=np.float32)
    vm = np.asarray(inputs["value_momentum"], dtype=np.float32)
    # mom is zeros in this problem; fall back to a host-side add if it isn't
    if np.any(km):
        out_k = out_k + np.float32(MOMENTUM) * km
    if np.any(vm):
        out_v = out_v + np.float32(MOMENTUM) * vm
    return out_k, out_v


# revision 22
# speedup vs baseline: 1.0033x; 1.0033x over previous
"""Trainium2 Bass kernel for nn_MemoryWriter (scatter_memory).

Math (see reference):
    w        = where(gate > 0.01, gate * 0.1, 0)            [B]
    contrib  (q_a, v_a, w_a) scattered to slots top_indices[a, :]
    upd_k[s] = sum_j w_j q_j / (counts>0 ? counts : 1), counts = sum_j w_j
    out_k    = mem_k + 0.9 * mom_k + (1 - 0.9) * upd_k      (mom is zeros)

Because upd is a ratio, the 0.1 UPDATE_RATE cancels between numerator and
denominator; we use raw gated gate values g = gate * (gate > 0.01) as weights
and apply the single (1 - momentum) factor at the end.  counts are either 0
or >= 0.01, and a zero count implies an exactly-zero numerator, so the
denominator select becomes rec01 = 1 / (max(counts, tiny) / (1-momentum)).

Sharding: slot dimension across 8 cores (8192 slots each).  The host performs
the contribution routing that the all-to-all performs in a real distributed
setting (the sharding hint: "route each (query, slot_idx) contribution to the
owning device (all-to-all on flattened top_indices)"): each core receives a
dense buffer of its routed contribution rows, packed [q | v | 1 | 1], grouped
by 128-slot tile.  Tiles are padded to a 32-row granularity and grouped into
capacity classes so the padding stays small.  The device then, per slot tile:
  - builds a weighted one-hot lhsT on the fly: (iota == s) * w, with s = -1
    sentinel on padding rows,
  - one PE float32r matmul per (tile, fragment) incidence accumulates
    [K-upd | V-upd | counts | counts] into a per-tile PSUM slice,
  - the ACT engine scales by (1-momentum)/counts, and DVE/GpSimd add the
    memory-table tile.
"""

import numpy as np

# ---- problem constants (hardcoded per contest contract) --------------------
N_SLOTS = 65536
DIM = 128
B = 4096
K = 8
NCORES = 8
SPC = N_SLOTS // NCORES      # slots per core = 8192
NT = SPC // 128              # slot tiles per core = 64
P = 128
EL = 258                     # packed row: [q(128) | v(128) | 1 | 1] f32
GATE_THRESH = 0.01
MOMENTUM = 0.9
UPD = float(np.float32(1.0) - np.float32(MOMENTUM))  # exactly as fp32 computes it
INV_UPD = float(np.float32(1.0) / np.float32(UPD))
USE_BF16 = True              # bf16 contribution path (1 cyc/row matmul)
USE_F32R = not USE_BF16      # float32r matmul (1 cyc/row at even N>=256)

_BUILD_CACHE = {}


def build_nc(struct):
    """Build the per-core Bass program.

    struct: (classes, incid) where classes is a tuple of
    (cap, ntiles, row_offset) DMA groups of the routed buffer and incid is a
    per slot-tile tuple of (col, class_id, pos, cap, start, stop) incidences.
    """
    import concourse.bacc as bacc
    import concourse.tile as tile
    from concourse import mybir
    from contextlib import ExitStack

    classes, incid = struct
    f32 = mybir.dt.float32
    f32r = mybir.dt.float32r
    Alu = mybir.AluOpType
    Act = mybir.ActivationFunctionType

    NCOL = sum(len(v) for v in incid)
    TOTROWS = sum(cap * nt for cap, nt, _ in classes)
    mmdt = mybir.dt.float16 if USE_BF16 else (f32r if USE_F32R else f32)

    nc = bacc.Bacc("TRN2", target_bir_lowering=False, debug=False)

    mem_kv = nc.dram_tensor("mem_kv", [SPC, 2 * DIM], f32, kind="ExternalInput")
    routed = nc.dram_tensor("routed", [TOTROWS, EL], mmdt, kind="ExternalInput")
    sv = nc.dram_tensor("sv", [P, NCOL], f32, kind="ExternalInput")
    wb = nc.dram_tensor("wb", [P, NCOL], f32, kind="ExternalInput")
    out_kv = nc.dram_tensor("out_kv", [SPC, 2 * DIM], f32, kind="ExternalOutput")

    G = 8                    # slot tiles per DMA group (512KB per table)
    PG = 4                   # slot tiles per PSUM group (4 banks)

    with tile.TileContext(nc) as tc, ExitStack() as ctx:
        const = ctx.enter_context(tc.tile_pool(name="const", bufs=1))
        gpool = ctx.enter_context(tc.tile_pool(name="gath", bufs=1))
        wpool = ctx.enter_context(tc.tile_pool(name="work", bufs=8))
        spool = ctx.enter_context(tc.tile_pool(name="small", bufs=8))
        upool = ctx.enter_context(tc.tile_pool(name="upd", bufs=6))
        pspool = ctx.enter_context(tc.tile_pool(name="ps", bufs=2, space="PSUM"))

        # constants / routing metadata
        iota_t = const.tile([P, 128], f32)
        nc.gpsimd.iota(
            iota_t[:], pattern=[[1, 128]], channel_multiplier=0,
            allow_small_or_imprecise_dtypes=True,
        )
        sv_t = const.tile([P, NCOL], f32)
        nc.sync.dma_start(sv_t[:], sv[:, :])
        wb_t = const.tile([P, NCOL], f32)
        nc.sync.dma_start(wb_t[:], wb[:, :])

        # w = gate * (gate > 0.01), per fragment column
        msk_t = const.tile([P, NCOL], f32)
        nc.vector.tensor_scalar(msk_t[:], wb_t[:], GATE_THRESH, None, op0=Alu.is_gt)
        w_t = const.tile([P, NCOL], f32)
        nc.vector.tensor_tensor(w_t[:], wb_t[:], msk_t[:], op=Alu.mult)

        # routed contribution rows, by capacity class.  Chunked loads so
        # compute can start before the whole buffer lands.
        clsbuf = []
        for ci, (cap, ntl, roff) in enumerate(classes):
            buf = gpool.tile([P, ntl * EL], mmdt, tag=f"cls{ci}")
            b3 = buf[:].rearrange("p (t e) -> p t e", e=EL)
            CH = max(1, (8 * 128) // cap)       # ~1K rows per chunk
            pos = 0
            while pos < ntl:
                bs = min(CH, ntl - pos)
                src = routed[roff + pos * cap: roff + (pos + bs) * cap, :]
                nc.sync.dma_start(
                    b3[0:cap, pos:pos + bs, :],
                    src.rearrange("(t p) e -> p t e", p=cap),
                )
                pos += bs
            clsbuf.append(b3)

        NPG = NT // PG
        for pg in range(NPG):
            r0 = pg * PG * 128
            ps = pspool.tile([P, PG * 512], f32, tag="ps")
            ps3 = ps[:].rearrange("p (i c) -> p i c", c=512)
            for i in range(PG):
                t = pg * PG + i
                for col, ci, tpos, cap, st, sp in incid[t]:
                    oh = wpool.tile([P, 128], mmdt, tag="oh")
                    nc.vector.tensor_scalar(
                        oh[0:cap, :], iota_t[0:cap, :],
                        sv_t[0:cap, col:col + 1], w_t[0:cap, col:col + 1],
                        op0=Alu.is_equal, op1=Alu.mult,
                    )
                    nc.tensor.matmul(
                        ps[:, i * 512:i * 512 + EL],
                        lhsT=oh[0:cap, :],
                        rhs=clsbuf[ci][0:cap, tpos, :],
                        start=st, stop=sp,
                    )
            # epilogue: counts are either 0 or >= 0.01; a zero count implies
            # an exactly-zero numerator, so clamp the denominator instead of
            # selecting: rec01 = 1 / (max(cnt, tiny) / UPD).
            cnt = ps3[:, :, 256:257]                      # [P, 4, 1]
            den = spool.tile([P, PG], f32, tag="den")
            nc.vector.tensor_scalar(den[:], cnt, 1e-30, INV_UPD,
                                    op0=Alu.max, op1=Alu.mult)
            rec01 = spool.tile([P, PG], f32, tag="rec01")
            nc.vector.reciprocal(rec01[:], den[:])

            # upd = psum * rec01 (per-partition scale), spread across engines
            upd = upool.tile([P, PG * 256], f32, tag="upd")
            upd3 = upd[:].rearrange("p (i c) -> p i c", c=256)
            for i in range(PG):
                if i < 2:
                    nc.scalar.activation(
                        upd3[:, i, :], ps3[:, i, 0:256], Act.Copy,
                        scale=rec01[:, i:i + 1],
                    )
                else:
                    nc.vector.tensor_scalar(
                        upd3[:, i, :], ps3[:, i, 0:256],
                        rec01[:, i:i + 1], None, op0=Alu.mult,
                    )

            # memory-table add rides the DMA (SWDGE accumulate), then store
            mkv = mem_kv[r0:r0 + PG * 128, :].rearrange("(a p) d -> p a d", p=P)
            okv = out_kv[r0:r0 + PG * 128, :].rearrange("(a p) d -> p a d", p=P)
            nc.gpsimd.dma_start(upd3[:, :, :], mkv, accum_op=Alu.add)
            nc.sync.dma_start(okv, upd3[:, :, :])

    nc.compile()
    return nc


def prepare_inputs(inputs):
    """Host-side routing (the all-to-all stand-in): bucket contributions by
    (core, slot-tile) and materialize each core's routed row buffer."""
    mkv = np.concatenate([
        np.asarray(inputs["memory_keys"], dtype=np.float32),
        np.asarray(inputs["memory_values"], dtype=np.float32),
    ], axis=1)
    q = np.asarray(inputs["write_query"], dtype=np.float32)
    v = np.asarray(inputs["write_value"], dtype=np.float32)
    gate = np.asarray(inputs["gate_weights"], dtype=np.float32)
    ti = np.asarray(inputs["top_indices"]).astype(np.int64).reshape(-1)

    qv = np.zeros((B, EL), dtype=np.float32)
    qv[:, 0:DIM] = q
    qv[:, DIM:2 * DIM] = v
    qv[:, 2 * DIM] = 1.0
    qv[:, 2 * DIM + 1] = 1.0   # second ones column: fp32r needs even width

    a = np.arange(B * K, dtype=np.int64) // K
    key = ti >> 7                       # global 128-slot tile id [0, 512)
    order = np.argsort(key, kind="stable")
    ks = key[order]
    a_s = a[order]
    s_s = (ti & 127)[order].astype(np.float32)
    cnt = np.bincount(key, minlength=NCORES * NT)
    starts = np.zeros(NCORES * NT + 1, dtype=np.int64)
    starts[1:] = np.cumsum(cnt)

    # Shared structure: per tile, fragments of <=128 rows sized by the max
    # count across cores, rounded up to 32-row granularity and grouped into
    # capacity classes.
    cnt2 = cnt.reshape(NCORES, NT)
    cnt_max = cnt2.max(axis=0)
    frags = []                          # (tile, frag_idx, cap)
    for t in range(NT):
        n = int(cnt_max[t])
        fi = 0
        while n > 128:
            frags.append((t, fi, 128))
            n -= 128
            fi += 1
        frags.append((t, fi, max(32, -(-n // 32) * 32)))

    caps = sorted({cap for _, _, cap in frags})
    classes = []
    frag_place = {}                     # (tile, fi) -> (col, ci, pos, cap)
    col = 0
    roff = 0
    for ci, cap in enumerate(caps):
        members = [f for f in frags if f[2] == cap]
        for pos, (t, fi, _) in enumerate(members):
            frag_place[(t, fi)] = (col, ci, pos, cap)
            col += 1
        classes.append((cap, len(members), roff))
        roff += cap * len(members)
    ncol = col
    totrows = roff

    incid = []
    for t in range(NT):
        lst = sorted(
            [v2 for (tt, fi), v2 in frag_place.items() if tt == t],
            key=lambda x: x[0],
        )
        n = len(lst)
        incid.append(tuple(
            (c, ci, pos, cap, i == 0, i == n - 1)
            for i, (c, ci, pos, cap) in enumerate(lst)
        ))
    incid = tuple(incid)
    struct = (tuple(classes), incid)

    if USE_BF16:
        qv = qv.astype(np.float16)
    in_maps = []
    for c in range(NCORES):
        routed = np.zeros((totrows, EL), dtype=qv.dtype)
        sv_core = np.full((P, ncol), -1.0, dtype=np.float32)
        wb_core = np.zeros((P, ncol), dtype=np.float32)
        for t in range(NT):
            n_c = int(cnt2[c, t])
            src0 = int(starts[c * NT + t])
            done = 0
            for (cc, ci, pos, cap, st, sp) in incid[t]:
                take = min(cap, n_c - done)
                if take <= 0:
                    break
                rows = slice(src0 + done, src0 + done + take)
                cap_, ntl_, roff_ = classes[ci]
                base = roff_ + pos * cap
                routed[base:base + take] = qv[a_s[rows]]
                prt = np.arange(0, take)
                sv_core[prt, cc] = s_s[rows]
                wb_core[prt, cc] = gate[a_s[rows]]
                done += take
        in_maps.append({
            "mem_kv": mkv[c * SPC:(c + 1) * SPC],
            "routed": routed,
            "sv": np.ascontiguousarray(sv_core),
            "wb": np.ascontiguousarray(wb_core),
        })
    return in_maps, struct


def kernel(**inputs):
    from concourse.bass_utils import run_bass_kernel_spmd

    in_maps, struct = prepare_inputs(inputs)
    if struct not in _BUILD_CACHE:
        _BUILD_CACHE[struct] = build_nc(struct)
    nc = _BUILD_CACHE[struct]

    res = run_bass_kernel_spmd(nc, in_maps, core_ids=list(range(NCORES)))
    out_kv = np.concatenate([res.results[c]["out_kv"] for c in range(NCORES)], axis=0)
    out_k = np.ascontiguousarray(out_kv[:, 0:DIM])
    out_v = np.ascontiguousarray(out_kv[:, DIM:2 * DIM])

    km = np.asarray(inputs["key_momentum"], dtype=np.float32)
    vm = np.asarray(inputs["value_momentum"], dtype=np.float32)
    # mom is zeros in this problem; fall back to a host-side add if it isn't
    if np.any(km):
        out_k = out_k + np.float32(MOMENTUM) * km
    if np.any(vm):
        out_v = out_v + np.float32(MOMENTUM) * vm
    return out_k, out_v
